# revision 12
# baseline (speedup 1.0000x reference)
"""Trainium2 Bass kernel for a 2-layer LSTM (64, 32) + MLP head.

Model (PyTorch semantics, eval mode):
    h1 = LSTM(4 -> 64)(x)            x: [B=4096, T=512, 4]
    h2 = LSTM(64 -> 32)(h1)
    y  = (relu(h2[:, -1] @ w_fc1.T + b_fc1)) @ w_fc2.T + b_fc2   # [B, 1]

Sharding: data-parallel over batch across 8 NeuronCores (512 rows each),
weights replicated. Inside each core the state is kept *transposed*
([units, batch]) so the per-timestep recurrent matmuls have batch on the
moving free dimension (N=512) and the gate nonlinearities run as a few
wide ops on full 96-partition stacks (layer-1 and layer-2 gates stacked).

State tile S [97, 512]: rows 0:64 = h1^T, rows 64:96 = h2^T, row 96 =
ones (bias row).  Both layers' recurrent matmuls use rhs S[0:97] (base
partition 0 — the PE moving operand must start at 0 to span >32
partitions); layer-1's weight rows over the h2 region are zeros, and
layer-2's over nothing (it genuinely uses h1+h2).  Biases ride the
ones-row through the matmul (incl. the fc1 bias in the head).
The input projection is a separate K=4 matmul per gate accumulating into
the same PSUM bank; x_t arrives per step by DMA into a small [4, 512]
rotating tile (x is recurrence-independent, so these prefetch ahead).

PSUM gate tile P [96, 2048] (4 banks): free slices i@0, f@512, o@1024,
g@1536; partitions 0:64 = layer-1 gate, 64:96 = layer-2 gate.  Sigmoid
is then ONE activation op over [96, 1536] (i,f,o) and tanh one op over
[96, 512] (g); the cell/hidden updates are [96, 512] vector ops.
"""

import numpy as np
from contextlib import ExitStack

import concourse.bass as bass
import concourse.tile as tile
from concourse import bacc, mybir
from concourse import bass_utils

AF = mybir.ActivationFunctionType

B, T, D_IN, H1, H2 = 4096, 512, 4, 64, 32
NCORES = 8
BL = B // NCORES  # 512 batch rows per core

F32 = mybir.dt.float32
# Compute dtypes (flip for perf/accuracy trades):
DT = mybir.dt.float32   # weights / state / gate-activation dtype
CDT = mybir.dt.float32  # cell-state dtype

HS = H1 + H2  # 96: stacked (layer1, layer2) partition extent


def _build(n_steps: int = T):
    """Build the SPMD single-core Bass program (same NEFF on all 8 cores)."""
    nc = bacc.Bacc("TRN2", target_bir_lowering=False, debug=False)

    xT = nc.dram_tensor("xT", [n_steps * 4, BL], DT, kind="ExternalInput")
    w1t = nc.dram_tensor("w1t", [97, 4 * H1], DT, kind="ExternalInput")
    w1x = nc.dram_tensor("w1x", [4, 4 * H1], DT, kind="ExternalInput")
    w2t = nc.dram_tensor("w2t", [97, 4 * H2], DT, kind="ExternalInput")
    wf1 = nc.dram_tensor("wf1", [97, 16], DT, kind="ExternalInput")
    wf2 = nc.dram_tensor("wf2", [16, 1], DT, kind="ExternalInput")
    bf2 = nc.dram_tensor("bf2", [1, 1], F32, kind="ExternalInput")
    out = nc.dram_tensor("out", [1, BL], F32, kind="ExternalOutput")

    with tile.TileContext(nc) as tc, ExitStack() as ctx:
        const = ctx.enter_context(tc.tile_pool(name="const", bufs=1))
        xpool = ctx.enter_context(tc.tile_pool(name="xp", bufs=8))
        gates = ctx.enter_context(tc.tile_pool(name="gates", bufs=3))

        W1 = const.tile([97, 4 * H1], DT, tag="W1")
        nc.sync.dma_start(W1[:], w1t.ap())
        W1X = const.tile([4, 4 * H1], DT, tag="W1X")
        nc.sync.dma_start(W1X[:], w1x.ap())
        W2 = const.tile([97, 4 * H2], DT, tag="W2")
        nc.sync.dma_start(W2[:], w2t.ap())
        WF1 = const.tile([97, 16], DT, tag="WF1")
        nc.sync.dma_start(WF1[:], wf1.ap())
        WF2 = const.tile([16, 1], DT, tag="WF2")
        nc.sync.dma_start(WF2[:], wf2.ap())
        BF2 = const.tile([1, 1], F32, tag="BF2")
        nc.sync.dma_start(BF2[:], bf2.ap())

        S = const.tile([97, BL], DT, tag="S")
        C = const.tile([HS, BL], CDT, tag="C")
        nc.vector.memset(S[:], 0.0)
        nc.vector.memset(S[96:97, :], 1.0)
        nc.vector.memset(C[:], 0.0)

        # gate -> PSUM free offset: i@0, f@512, o@1024, g@1536
        # (PyTorch gate packing order in the weight columns is i,f,g,o.)
        mm_order = ((2, 1536), (0, 0), (1, 512), (3, 1024))  # g first

        # Software-pipelined over layers: at iteration k the layer-1
        # partition (rows 0:64) computes LSTM-1 step k while the layer-2
        # partition (rows 64:96) computes LSTM-2 step k-1 — both read
        # h1_{k-1} from S, which is exactly what each needs.  Iteration 0
        # produces garbage layer-2 state (cleared right after); iteration
        # n_steps produces garbage layer-1 state (never consumed: the head
        # weights are zero over the h1 rows).
        with tc.tile_pool(name="psum", bufs=2, space="PSUM") as psum:
            for k in range(n_steps + 1):
                do_x = k < n_steps
                if do_x:
                    XTT = xpool.tile([4, BL], DT, tag="xt")
                    nc.sync.dma_start(XTT[:], xT.ap()[4 * k : 4 * k + 4, :])
                P = psum.tile([HS, 2048], F32, tag="P")
                for gsel, boff in mm_order:
                    nc.tensor.matmul(
                        P[0:H1, boff : boff + BL],
                        W1[:, gsel * H1 : (gsel + 1) * H1],
                        S[0:97, :],
                        start=True,
                        stop=not do_x,
                    )
                    if do_x:
                        nc.tensor.matmul(
                            P[0:H1, boff : boff + BL],
                            W1X[:, gsel * H1 : (gsel + 1) * H1],
                            XTT[:],
                            start=False,
                            stop=True,
                        )
                    nc.tensor.matmul(
                        P[H1:HS, boff : boff + BL],
                        W2[:, gsel * H2 : (gsel + 1) * H2],
                        S[0:97, :],
                        start=True,
                        stop=True,
                    )

                G = gates.tile([HS, BL], DT, tag="G")
                SIG = gates.tile([HS, 3 * BL], DT, tag="SIG")
                nc.scalar.activation(G[:], P[:, 1536:2048], AF.Tanh)
                nc.scalar.activation(SIG[:], P[:, 0:1536], AF.Sigmoid)

                U = gates.tile([HS, BL], DT, tag="U")
                V = gates.tile([HS, BL], CDT, tag="V")
                nc.vector.tensor_mul(U[:], SIG[:, 0:BL], G[:])          # i*g
                nc.vector.tensor_mul(V[:], SIG[:, BL : 2 * BL], C[:])   # f*c
                nc.vector.tensor_add(C[:], U[:], V[:])                  # c'
                TC = gates.tile([HS, BL], DT, tag="TC")
                nc.scalar.activation(TC[:], C[:], AF.Tanh)
                nc.vector.tensor_mul(S[0:HS, :], SIG[:, 2 * BL :], TC[:])  # h
                if k == 0:
                    # wipe the garbage layer-2 state from the pipeline warmup
                    nc.vector.memset(S[H1:HS, :], 0.0)
                    nc.vector.memset(C[H1:HS, :], 0.0)

        # MLP head on h2 at the last timestep (rows 64:96 of S).
        with tc.tile_pool(name="psum_head", bufs=1, space="PSUM") as psh:
            PF = psh.tile([16, BL], F32, tag="PF")
            nc.tensor.matmul(PF[:], WF1[:, :], S[0:97, :], start=True, stop=True)
            Z = gates.tile([16, BL], DT, tag="Z")
            nc.scalar.activation(Z[:], PF[:], AF.Relu)
            PO = psh.tile([1, BL], F32, tag="PO")
            nc.tensor.matmul(PO[:], WF2[:, :], Z[:], start=True, stop=True)
            Y = gates.tile([1, BL], F32, tag="Y")
            nc.scalar.activation(Y[:], PO[:], AF.Identity, bias=BF2[:, 0:1])
            nc.sync.dma_start(out.ap(), Y[:])

    nc.compile()
    return nc


def _pack_weights(inputs, np_dt):
    w_ih1, w_hh1 = inputs["w_ih1"], inputs["w_hh1"]
    w_ih2, w_hh2 = inputs["w_ih2"], inputs["w_hh2"]
    b1 = (inputs["b_ih1"] + inputs["b_hh1"]).astype(np.float32)
    b2 = (inputs["b_ih2"] + inputs["b_hh2"]).astype(np.float32)
    # [97, 256]: rows = [w_hh1^T(64); zeros(32); bias1(1)] matching rhs
    # S[0:97] = [h1; h2(ignored); ones]
    z32 = np.zeros((4 * 64, 32), np.float32)
    w1t = np.concatenate([w_hh1, z32, b1[:, None]], axis=1).T
    # [97, 128]: rows = [w_ih2^T(64); w_hh2^T(32); bias2(1)] matching rhs
    # S[0:97] = [h1; h2; ones]
    w2t = np.concatenate([w_ih2, w_hh2, b2[:, None]], axis=1).T
    return {
        "w1t": np.ascontiguousarray(w1t).astype(np_dt),
        "w1x": np.ascontiguousarray(w_ih1.T).astype(np_dt),
        "w2t": np.ascontiguousarray(w2t).astype(np_dt),
        "wf1": np.ascontiguousarray(np.concatenate(
            [np.zeros((64, 16), np.float32), inputs["w_fc1"].T,
             inputs["b_fc1"][None, :]], axis=0)).astype(np_dt),
        "wf2": np.ascontiguousarray(inputs["w_fc2"].T).astype(np_dt),
        "bf2": np.ascontiguousarray(inputs["b_fc2"][:, None]).astype(np.float32),
    }


_built = {}


def _get_nc(n_steps):
    if n_steps not in _built:
        _built[n_steps] = _build(n_steps)
    return _built[n_steps]


def _run(inputs, n_steps=T, **run_kwargs):
    np_dt = mybir.dt.np(DT)
    x = np.asarray(inputs["x"], np.float32)
    nb = x.shape[0]
    ncores = NCORES
    bl = nb // ncores
    assert bl == BL and x.shape[1] >= n_steps
    shared = _pack_weights({k: np.asarray(v, np.float32) for k, v in inputs.items()
                            if k != "x"} | {}, np_dt)
    in_maps = []
    for c in range(ncores):
        xs = x[c * bl : (c + 1) * bl, :n_steps, :]  # [BL, T, 4]
        xT = np.ascontiguousarray(xs.transpose(1, 2, 0).reshape(n_steps * 4, bl))
        in_maps.append(dict(shared, xT=xT.astype(np_dt)))
    nc = _get_nc(n_steps)
    res = bass_utils.run_bass_kernel_spmd(
        nc, in_maps, core_ids=list(range(ncores)), **run_kwargs
    )
    y = np.concatenate(
        [np.asarray(r["out"], np.float32).reshape(bl, 1) for r in res.results], axis=0
    )
    return y, res


def kernel(**inputs) -> np.ndarray:
    y, _ = _run(inputs)
    return y


# revision 13
# speedup vs baseline: 2.3511x; 2.3511x over previous
"""Trainium2 Bass kernel for a 2-layer LSTM (64, 32) + MLP head.

Model (PyTorch semantics, eval mode):
    h1 = LSTM(4 -> 64)(x)            x: [B=4096, T=512, 4]
    h2 = LSTM(64 -> 32)(h1)
    y  = (relu(h2[:, -1] @ w_fc1.T + b_fc1)) @ w_fc2.T + b_fc2   # [B, 1]

Sharding: data-parallel over batch across 8 NeuronCores (512 rows each),
weights replicated. Inside each core the state is kept *transposed*
([units, batch]) so the per-timestep recurrent matmuls have batch on the
moving free dimension (N=512) and the gate nonlinearities run as a few
wide ops on full 96-partition stacks (layer-1 and layer-2 gates stacked).

State tile S [97, 512]: rows 0:64 = h1^T, rows 64:96 = h2^T, row 96 =
ones (bias row).  Both layers' recurrent matmuls use rhs S[0:97] (base
partition 0 — the PE moving operand must start at 0 to span >32
partitions); layer-1's weight rows over the h2 region are zeros, and
layer-2's over nothing (it genuinely uses h1+h2).  Biases ride the
ones-row through the matmul (incl. the fc1 bias in the head).
The input projection is a separate K=4 matmul per gate accumulating into
the same PSUM bank; x_t arrives per step by DMA into a small [4, 512]
rotating tile (x is recurrence-independent, so these prefetch ahead).

PSUM gate tile P [96, 2048] (4 banks): free slices i@0, f@512, o@1024,
g@1536; partitions 0:64 = layer-1 gate, 64:96 = layer-2 gate.  Sigmoid
is then ONE activation op over [96, 1536] (i,f,o) and tanh one op over
[96, 512] (g); the cell/hidden updates are [96, 512] vector ops.
"""

import numpy as np
from contextlib import ExitStack

import concourse.bass as bass
import concourse.tile as tile
from concourse import bacc, mybir
from concourse import bass_utils

AF = mybir.ActivationFunctionType

B, T, D_IN, H1, H2 = 4096, 512, 4, 64, 32
NCORES = 8
BL = B // NCORES  # 512 batch rows per core

F32 = mybir.dt.float32
# Compute dtypes (flip for perf/accuracy trades):
DT = mybir.dt.bfloat16  # weights / state / gate-activation dtype
CDT = mybir.dt.float32  # cell-state dtype

HS = H1 + H2  # 96: stacked (layer1, layer2) partition extent


def _build(n_steps: int = T):
    """Build the SPMD single-core Bass program (same NEFF on all 8 cores)."""
    nc = bacc.Bacc("TRN2", target_bir_lowering=False, debug=False)

    xT = nc.dram_tensor("xT", [n_steps * 4, BL], DT, kind="ExternalInput")
    w1t = nc.dram_tensor("w1t", [97, 4 * H1], DT, kind="ExternalInput")
    w1x = nc.dram_tensor("w1x", [4, 4 * H1], DT, kind="ExternalInput")
    w2t = nc.dram_tensor("w2t", [97, 4 * H2], DT, kind="ExternalInput")
    wf1 = nc.dram_tensor("wf1", [97, 16], DT, kind="ExternalInput")
    wf2 = nc.dram_tensor("wf2", [16, 1], DT, kind="ExternalInput")
    bf2 = nc.dram_tensor("bf2", [1, 1], F32, kind="ExternalInput")
    out = nc.dram_tensor("out", [1, BL], F32, kind="ExternalOutput")

    with tile.TileContext(nc) as tc, ExitStack() as ctx:
        const = ctx.enter_context(tc.tile_pool(name="const", bufs=1))
        xpool = ctx.enter_context(tc.tile_pool(name="xp", bufs=8))
        gates = ctx.enter_context(tc.tile_pool(name="gates", bufs=3))

        W1 = const.tile([97, 4 * H1], DT, tag="W1")
        nc.sync.dma_start(W1[:], w1t.ap())
        W1X = const.tile([4, 4 * H1], DT, tag="W1X")
        nc.sync.dma_start(W1X[:], w1x.ap())
        W2 = const.tile([97, 4 * H2], DT, tag="W2")
        nc.sync.dma_start(W2[:], w2t.ap())
        WF1 = const.tile([97, 16], DT, tag="WF1")
        nc.sync.dma_start(WF1[:], wf1.ap())
        WF2 = const.tile([16, 1], DT, tag="WF2")
        nc.sync.dma_start(WF2[:], wf2.ap())
        BF2 = const.tile([1, 1], F32, tag="BF2")
        nc.sync.dma_start(BF2[:], bf2.ap())

        S = const.tile([97, BL], DT, tag="S")
        C = const.tile([HS, BL], CDT, tag="C")
        nc.vector.memset(S[:], 0.0)
        nc.vector.memset(S[96:97, :], 1.0)
        nc.vector.memset(C[:], 0.0)

        # gate -> PSUM free offset: i@0, f@512, o@1024, g@1536
        # (PyTorch gate packing order in the weight columns is i,f,g,o.)
        mm_order = ((2, 1536), (0, 0), (1, 512), (3, 1024))  # g first

        # Software-pipelined over layers: at iteration k the layer-1
        # partition (rows 0:64) computes LSTM-1 step k while the layer-2
        # partition (rows 64:96) computes LSTM-2 step k-1 — both read
        # h1_{k-1} from S, which is exactly what each needs.  Iteration 0
        # produces garbage layer-2 state (cleared right after); iteration
        # n_steps produces garbage layer-1 state (never consumed: the head
        # weights are zero over the h1 rows).
        with tc.tile_pool(name="psum", bufs=2, space="PSUM") as psum:
            for k in range(n_steps + 1):
                do_x = k < n_steps
                if do_x:
                    XTT = xpool.tile([4, BL], DT, tag="xt")
                    nc.sync.dma_start(XTT[:], xT.ap()[4 * k : 4 * k + 4, :])
                P = psum.tile([HS, 2048], F32, tag="P")
                for gsel, boff in mm_order:
                    nc.tensor.matmul(
                        P[0:H1, boff : boff + BL],
                        W1[:, gsel * H1 : (gsel + 1) * H1],
                        S[0:97, :],
                        start=True,
                        stop=not do_x,
                    )
                    if do_x:
                        nc.tensor.matmul(
                            P[0:H1, boff : boff + BL],
                            W1X[:, gsel * H1 : (gsel + 1) * H1],
                            XTT[:],
                            start=False,
                            stop=True,
                        )
                    nc.tensor.matmul(
                        P[H1:HS, boff : boff + BL],
                        W2[:, gsel * H2 : (gsel + 1) * H2],
                        S[0:97, :],
                        start=True,
                        stop=True,
                    )

                G = gates.tile([HS, BL], DT, tag="G")
                SIG = gates.tile([HS, 3 * BL], DT, tag="SIG")
                nc.scalar.activation(G[:], P[:, 1536:2048], AF.Tanh)
                nc.scalar.activation(SIG[:], P[:, 0:1536], AF.Sigmoid)

                U = gates.tile([HS, BL], DT, tag="U")
                V = gates.tile([HS, BL], CDT, tag="V")
                nc.vector.tensor_mul(U[:], SIG[:, 0:BL], G[:])          # i*g
                nc.vector.tensor_mul(V[:], SIG[:, BL : 2 * BL], C[:])   # f*c
                nc.vector.tensor_add(C[:], U[:], V[:])                  # c'
                TC = gates.tile([HS, BL], DT, tag="TC")
                nc.scalar.activation(TC[:], C[:], AF.Tanh)
                nc.vector.tensor_mul(S[0:HS, :], SIG[:, 2 * BL :], TC[:])  # h
                if k == 0:
                    # wipe the garbage layer-2 state from the pipeline warmup
                    nc.vector.memset(S[H1:HS, :], 0.0)
                    nc.vector.memset(C[H1:HS, :], 0.0)

        # MLP head on h2 at the last timestep (rows 64:96 of S).
        with tc.tile_pool(name="psum_head", bufs=1, space="PSUM") as psh:
            PF = psh.tile([16, BL], F32, tag="PF")
            nc.tensor.matmul(PF[:], WF1[:, :], S[0:97, :], start=True, stop=True)
            Z = gates.tile([16, BL], DT, tag="Z")
            nc.scalar.activation(Z[:], PF[:], AF.Relu)
            PO = psh.tile([1, BL], F32, tag="PO")
            nc.tensor.matmul(PO[:], WF2[:, :], Z[:], start=True, stop=True)
            Y = gates.tile([1, BL], F32, tag="Y")
            nc.scalar.activation(Y[:], PO[:], AF.Identity, bias=BF2[:, 0:1])
            nc.sync.dma_start(out.ap(), Y[:])

    nc.compile()
    return nc


def _pack_weights(inputs, np_dt):
    w_ih1, w_hh1 = inputs["w_ih1"], inputs["w_hh1"]
    w_ih2, w_hh2 = inputs["w_ih2"], inputs["w_hh2"]
    b1 = (inputs["b_ih1"] + inputs["b_hh1"]).astype(np.float32)
    b2 = (inputs["b_ih2"] + inputs["b_hh2"]).astype(np.float32)
    # [97, 256]: rows = [w_hh1^T(64); zeros(32); bias1(1)] matching rhs
    # S[0:97] = [h1; h2(ignored); ones]
    z32 = np.zeros((4 * 64, 32), np.float32)
    w1t = np.concatenate([w_hh1, z32, b1[:, None]], axis=1).T
    # [97, 128]: rows = [w_ih2^T(64); w_hh2^T(32); bias2(1)] matching rhs
    # S[0:97] = [h1; h2; ones]
    w2t = np.concatenate([w_ih2, w_hh2, b2[:, None]], axis=1).T
    return {
        "w1t": np.ascontiguousarray(w1t).astype(np_dt),
        "w1x": np.ascontiguousarray(w_ih1.T).astype(np_dt),
        "w2t": np.ascontiguousarray(w2t).astype(np_dt),
        "wf1": np.ascontiguousarray(np.concatenate(
            [np.zeros((64, 16), np.float32), inputs["w_fc1"].T,
             inputs["b_fc1"][None, :]], axis=0)).astype(np_dt),
        "wf2": np.ascontiguousarray(inputs["w_fc2"].T).astype(np_dt),
        "bf2": np.ascontiguousarray(inputs["b_fc2"][:, None]).astype(np.float32),
    }


_built = {}


def _get_nc(n_steps):
    if n_steps not in _built:
        _built[n_steps] = _build(n_steps)
    return _built[n_steps]


def _run(inputs, n_steps=T, **run_kwargs):
    np_dt = mybir.dt.np(DT)
    x = np.asarray(inputs["x"], np.float32)
    nb = x.shape[0]
    ncores = NCORES
    bl = nb // ncores
    assert bl == BL and x.shape[1] >= n_steps
    shared = _pack_weights({k: np.asarray(v, np.float32) for k, v in inputs.items()
                            if k != "x"} | {}, np_dt)
    in_maps = []
    for c in range(ncores):
        xs = x[c * bl : (c + 1) * bl, :n_steps, :]  # [BL, T, 4]
        xT = np.ascontiguousarray(xs.transpose(1, 2, 0).reshape(n_steps * 4, bl))
        in_maps.append(dict(shared, xT=xT.astype(np_dt)))
    nc = _get_nc(n_steps)
    res = bass_utils.run_bass_kernel_spmd(
        nc, in_maps, core_ids=list(range(ncores)), **run_kwargs
    )
    y = np.concatenate(
        [np.asarray(r["out"], np.float32).reshape(bl, 1) for r in res.results], axis=0
    )
    return y, res


def kernel(**inputs) -> np.ndarray:
    y, _ = _run(inputs)
    return y


# revision 15
# speedup vs baseline: 3.0959x; 1.3167x over previous
"""Trainium2 Bass kernel for a 2-layer LSTM (64, 32) + MLP head.

Model (PyTorch semantics, eval mode):
    h1 = LSTM(4 -> 64)(x)            x: [B=4096, T=512, 4]
    h2 = LSTM(64 -> 32)(h1)
    y  = (relu(h2[:, -1] @ w_fc1.T + b_fc1)) @ w_fc2.T + b_fc2   # [B, 1]

Sharding: data-parallel over batch across 8 NeuronCores (512 rows each),
weights replicated. Inside each core the state is kept *transposed*
([units, batch]) so the per-timestep recurrent matmuls have batch on the
moving free dimension (N=512) and the gate nonlinearities run as a few
wide ops on full 96-partition stacks (layer-1 and layer-2 gates stacked).

State tile S [97, 512]: rows 0:64 = h1^T, rows 64:96 = h2^T, row 96 =
ones (bias row).  Both layers' recurrent matmuls use rhs S[0:97] (base
partition 0 — the PE moving operand must start at 0 to span >32
partitions); layer-1's weight rows over the h2 region are zeros, and
layer-2's over nothing (it genuinely uses h1+h2).  Biases ride the
ones-row through the matmul (incl. the fc1 bias in the head).
The input projection is a separate K=4 matmul per gate accumulating into
the same PSUM bank; x_t arrives per step by DMA into a small [4, 512]
rotating tile (x is recurrence-independent, so these prefetch ahead).

PSUM gate tile P [96, 2048] (4 banks): free slices i@0, f@512, o@1024,
g@1536; partitions 0:64 = layer-1 gate, 64:96 = layer-2 gate.  Sigmoid
is then ONE activation op over [96, 1536] (i,f,o) and tanh one op over
[96, 512] (g); the cell/hidden updates are [96, 512] vector ops.
"""

import numpy as np
from contextlib import ExitStack

import concourse.bass as bass
import concourse.tile as tile
from concourse import bacc, mybir
from concourse import bass_utils

AF = mybir.ActivationFunctionType

B, T, D_IN, H1, H2 = 4096, 512, 4, 64, 32
NCORES = 8
BL = B // NCORES  # 512 batch rows per core

F32 = mybir.dt.float32
# Compute dtypes (flip for perf/accuracy trades):
DT = mybir.dt.bfloat16  # weights / state / gate-activation dtype
CDT = mybir.dt.bfloat16  # cell-state dtype

HS = H1 + H2  # 96: stacked (layer1, layer2) partition extent


def _build(n_steps: int = T):
    """Build the SPMD single-core Bass program (same NEFF on all 8 cores)."""
    nc = bacc.Bacc("TRN2", target_bir_lowering=False, debug=False)

    xT = nc.dram_tensor("xT", [n_steps * 4, BL], DT, kind="ExternalInput")
    w1t = nc.dram_tensor("w1t", [97, 4 * H1], DT, kind="ExternalInput")
    w1x = nc.dram_tensor("w1x", [4, 4 * H1], DT, kind="ExternalInput")
    w2t = nc.dram_tensor("w2t", [97, 4 * H2], DT, kind="ExternalInput")
    wf1 = nc.dram_tensor("wf1", [97, 16], DT, kind="ExternalInput")
    wf2 = nc.dram_tensor("wf2", [16, 1], DT, kind="ExternalInput")
    bf2 = nc.dram_tensor("bf2", [1, 1], F32, kind="ExternalInput")
    out = nc.dram_tensor("out", [1, BL], F32, kind="ExternalOutput")

    with tile.TileContext(nc) as tc, ExitStack() as ctx:
        const = ctx.enter_context(tc.tile_pool(name="const", bufs=1))
        xpool = ctx.enter_context(tc.tile_pool(name="xp", bufs=8))
        gates = ctx.enter_context(tc.tile_pool(name="gates", bufs=3))

        W1 = const.tile([97, 4 * H1], DT, tag="W1")
        nc.sync.dma_start(W1[:], w1t.ap())
        W1X = const.tile([4, 4 * H1], DT, tag="W1X")
        nc.sync.dma_start(W1X[:], w1x.ap())
        W2 = const.tile([97, 4 * H2], DT, tag="W2")
        nc.sync.dma_start(W2[:], w2t.ap())
        WF1 = const.tile([97, 16], DT, tag="WF1")
        nc.sync.dma_start(WF1[:], wf1.ap())
        WF2 = const.tile([16, 1], DT, tag="WF2")
        nc.sync.dma_start(WF2[:], wf2.ap())
        BF2 = const.tile([1, 1], F32, tag="BF2")
        nc.sync.dma_start(BF2[:], bf2.ap())

        S = const.tile([97, BL], DT, tag="S")
        C = const.tile([HS, BL], CDT, tag="C")
        nc.vector.memset(S[:], 0.0)
        nc.vector.memset(S[96:97, :], 1.0)
        nc.vector.memset(C[:], 0.0)

        # gate -> PSUM free offset: f@0, i@512, o@1024, g@1536
        # (PyTorch gate packing order in the weight columns is i,f,g,o.)
        # f first so sigmoid(f) can be computed (and f*c started) while
        # the remaining gate matmuls still stream; g last, its tanh
        # overlaps the sigmoids.
        mm_order = ((1, 0), (0, 512), (3, 1024), (2, 1536))

        def emit_x_mms(P_tile, step):
            """Input-projection matmuls for `step` into P_tile (start=True).

            These depend only on x (not on the recurrent state), so they are
            emitted at the tail of the previous iteration: they fill the
            TensorE pipe while that iteration's ACT/DVE chain runs.
            """
            XTT = xpool.tile([4, BL], DT, tag="xt")
            nc.sync.dma_start(XTT[:], xT.ap()[4 * step : 4 * step + 4, :])
            for gsel, boff in mm_order:
                nc.tensor.matmul(
                    P_tile[0:H1, boff : boff + BL],
                    W1X[:, gsel * H1 : (gsel + 1) * H1],
                    XTT[:],
                    start=True,
                    stop=False,
                )

        # Software-pipelined over layers: at iteration k the layer-1
        # partition (rows 0:64) computes LSTM-1 step k while the layer-2
        # partition (rows 64:96) computes LSTM-2 step k-1 — both read
        # h1_{k-1} from S, which is exactly what each needs.  Iteration 0
        # produces garbage layer-2 state (cleared right after); iteration
        # n_steps produces garbage layer-1 state (never consumed: the head
        # weights are zero over the h1 rows).
        with tc.tile_pool(name="psum", bufs=2, space="PSUM") as psum:
            P = psum.tile([HS, 2048], F32, tag="P")
            emit_x_mms(P, 0)
            for k in range(n_steps + 1):
                has_x = k < n_steps  # P already holds the x contribution
                for gsel, boff in mm_order:
                    nc.tensor.matmul(
                        P[0:H1, boff : boff + BL],
                        W1[:, gsel * H1 : (gsel + 1) * H1],
                        S[0:97, :],
                        start=not has_x,
                        stop=True,
                    )
                    nc.tensor.matmul(
                        P[H1:HS, boff : boff + BL],
                        W2[:, gsel * H2 : (gsel + 1) * H2],
                        S[0:97, :],
                        start=True,
                        stop=True,
                    )

                if k + 1 <= n_steps:
                    P_next = psum.tile([HS, 2048], F32, tag="P")
                    if k + 1 < n_steps:
                        emit_x_mms(P_next, k + 1)

                SIGF = gates.tile([HS, BL], DT, tag="SIGF")
                SIGIO = gates.tile([HS, 2 * BL], DT, tag="SIGIO")
                G = gates.tile([HS, BL], DT, tag="G")
                nc.scalar.activation(SIGF[:], P[:, 0:BL], AF.Sigmoid)
                nc.scalar.activation(SIGIO[:], P[:, BL : 3 * BL], AF.Sigmoid)
                nc.scalar.activation(G[:], P[:, 3 * BL :], AF.Tanh)

                U = gates.tile([HS, BL], DT, tag="U")
                V = gates.tile([HS, BL], CDT, tag="V")
                nc.vector.tensor_mul(V[:], SIGF[:], C[:])               # f*c
                nc.vector.tensor_mul(U[:], SIGIO[:, 0:BL], G[:])        # i*g
                nc.vector.tensor_add(C[:], U[:], V[:])                  # c'
                TC = gates.tile([HS, BL], DT, tag="TC")
                nc.scalar.activation(TC[:], C[:], AF.Tanh)
                nc.vector.tensor_mul(S[0:HS, :], SIGIO[:, BL:], TC[:])  # h
                if k == 0:
                    # wipe the garbage layer-2 state from the pipeline warmup
                    nc.vector.memset(S[H1:HS, :], 0.0)
                    nc.vector.memset(C[H1:HS, :], 0.0)
                if k + 1 <= n_steps:
                    P = P_next

        # MLP head on h2 at the last timestep (rows 64:96 of S).
        with tc.tile_pool(name="psum_head", bufs=1, space="PSUM") as psh:
            PF = psh.tile([16, BL], F32, tag="PF")
            nc.tensor.matmul(PF[:], WF1[:, :], S[0:97, :], start=True, stop=True)
            Z = gates.tile([16, BL], DT, tag="Z")
            nc.scalar.activation(Z[:], PF[:], AF.Relu)
            PO = psh.tile([1, BL], F32, tag="PO")
            nc.tensor.matmul(PO[:], WF2[:, :], Z[:], start=True, stop=True)
            Y = gates.tile([1, BL], F32, tag="Y")
            nc.scalar.activation(Y[:], PO[:], AF.Identity, bias=BF2[:, 0:1])
            nc.sync.dma_start(out.ap(), Y[:])

    nc.compile()
    return nc


def _pack_weights(inputs, np_dt):
    w_ih1, w_hh1 = inputs["w_ih1"], inputs["w_hh1"]
    w_ih2, w_hh2 = inputs["w_ih2"], inputs["w_hh2"]
    b1 = (inputs["b_ih1"] + inputs["b_hh1"]).astype(np.float32)
    b2 = (inputs["b_ih2"] + inputs["b_hh2"]).astype(np.float32)
    # [97, 256]: rows = [w_hh1^T(64); zeros(32); bias1(1)] matching rhs
    # S[0:97] = [h1; h2(ignored); ones]
    z32 = np.zeros((4 * 64, 32), np.float32)
    w1t = np.concatenate([w_hh1, z32, b1[:, None]], axis=1).T
    # [97, 128]: rows = [w_ih2^T(64); w_hh2^T(32); bias2(1)] matching rhs
    # S[0:97] = [h1; h2; ones]
    w2t = np.concatenate([w_ih2, w_hh2, b2[:, None]], axis=1).T
    return {
        "w1t": np.ascontiguousarray(w1t).astype(np_dt),
        "w1x": np.ascontiguousarray(w_ih1.T).astype(np_dt),
        "w2t": np.ascontiguousarray(w2t).astype(np_dt),
        "wf1": np.ascontiguousarray(np.concatenate(
            [np.zeros((64, 16), np.float32), inputs["w_fc1"].T,
             inputs["b_fc1"][None, :]], axis=0)).astype(np_dt),
        "wf2": np.ascontiguousarray(inputs["w_fc2"].T).astype(np_dt),
        "bf2": np.ascontiguousarray(inputs["b_fc2"][:, None]).astype(np.float32),
    }


_built = {}


def _get_nc(n_steps):
    if n_steps not in _built:
        _built[n_steps] = _build(n_steps)
    return _built[n_steps]


def _run(inputs, n_steps=T, **run_kwargs):
    np_dt = mybir.dt.np(DT)
    x = np.asarray(inputs["x"], np.float32)
    nb = x.shape[0]
    ncores = NCORES
    bl = nb // ncores
    assert bl == BL and x.shape[1] >= n_steps
    shared = _pack_weights({k: np.asarray(v, np.float32) for k, v in inputs.items()
                            if k != "x"} | {}, np_dt)
    in_maps = []
    for c in range(ncores):
        xs = x[c * bl : (c + 1) * bl, :n_steps, :]  # [BL, T, 4]
        xT = np.ascontiguousarray(xs.transpose(1, 2, 0).reshape(n_steps * 4, bl))
        in_maps.append(dict(shared, xT=xT.astype(np_dt)))
    nc = _get_nc(n_steps)
    res = bass_utils.run_bass_kernel_spmd(
        nc, in_maps, core_ids=list(range(ncores)), **run_kwargs
    )
    y = np.concatenate(
        [np.asarray(r["out"], np.float32).reshape(bl, 1) for r in res.results], axis=0
    )
    return y, res


def kernel(**inputs) -> np.ndarray:
    y, _ = _run(inputs)
    return y


# revision 16
# speedup vs baseline: 5.2170x; 1.6852x over previous
"""Trainium2 Bass kernel for a 2-layer LSTM (64, 32) + MLP head.

Model (PyTorch semantics, eval mode):
    h1 = LSTM(4 -> 64)(x)            x: [B=4096, T=512, 4]
    h2 = LSTM(64 -> 32)(h1)
    y  = (relu(h2[:, -1] @ w_fc1.T + b_fc1)) @ w_fc2.T + b_fc2   # [B, 1]

Sharding: data-parallel over batch across 8 NeuronCores (512 rows each),
weights replicated. Inside each core the state is kept *transposed*
([units, batch]) so the per-timestep recurrent matmuls have batch on the
moving free dimension (N=512) and the gate nonlinearities run as a few
wide ops on full 96-partition stacks (layer-1 and layer-2 gates stacked).

State tile S [97, 512]: rows 0:64 = h1^T, rows 64:96 = h2^T, row 96 =
ones (bias row).  Both layers' recurrent matmuls use rhs S[0:97] (base
partition 0 — the PE moving operand must start at 0 to span >32
partitions); layer-1's weight rows over the h2 region are zeros, and
layer-2's over nothing (it genuinely uses h1+h2).  Biases ride the
ones-row through the matmul (incl. the fc1 bias in the head).
The input projection is a separate K=4 matmul per gate accumulating into
the same PSUM bank; x_t arrives per step by DMA into a small [4, 512]
rotating tile (x is recurrence-independent, so these prefetch ahead).

PSUM gate tile P [96, 2048] (4 banks): free slices i@0, f@512, o@1024,
g@1536; partitions 0:64 = layer-1 gate, 64:96 = layer-2 gate.  Sigmoid
is then ONE activation op over [96, 1536] (i,f,o) and tanh one op over
[96, 512] (g); the cell/hidden updates are [96, 512] vector ops.
"""

import numpy as np
from contextlib import ExitStack

import concourse.bass as bass
import concourse.tile as tile
from concourse import bacc, mybir
from concourse import bass_utils

AF = mybir.ActivationFunctionType

B, T, D_IN, H1, H2 = 4096, 512, 4, 64, 32
NCORES = 8
BL = B // NCORES  # 512 batch rows per core

F32 = mybir.dt.float32
# Compute dtypes (flip for perf/accuracy trades):
DT = mybir.dt.bfloat16  # weights / state / gate-activation dtype
CDT = mybir.dt.bfloat16  # cell-state dtype

HS = H1 + H2  # 96: stacked (layer1, layer2) partition extent


def _build(n_steps: int = T):
    """Build the SPMD single-core Bass program (same NEFF on all 8 cores)."""
    nc = bacc.Bacc("TRN2", target_bir_lowering=False, debug=False)

    xT = nc.dram_tensor("xT", [n_steps * 4, BL], DT, kind="ExternalInput")
    w12t = nc.dram_tensor("w12t", [97, 4 * HS], DT, kind="ExternalInput")
    w1x = nc.dram_tensor("w1x", [4, 4 * HS], DT, kind="ExternalInput")
    wf1 = nc.dram_tensor("wf1", [97, 16], DT, kind="ExternalInput")
    wf2 = nc.dram_tensor("wf2", [16, 1], DT, kind="ExternalInput")
    bf2 = nc.dram_tensor("bf2", [1, 1], F32, kind="ExternalInput")
    out = nc.dram_tensor("out", [1, BL], F32, kind="ExternalOutput")

    with tile.TileContext(nc) as tc, ExitStack() as ctx:
        const = ctx.enter_context(tc.tile_pool(name="const", bufs=1))
        xpool = ctx.enter_context(tc.tile_pool(name="xp", bufs=8))
        gates = ctx.enter_context(tc.tile_pool(name="gates", bufs=3))

        W12 = const.tile([97, 4 * HS], DT, tag="W12")
        nc.sync.dma_start(W12[:], w12t.ap())
        W1X = const.tile([4, 4 * HS], DT, tag="W1X")
        nc.sync.dma_start(W1X[:], w1x.ap())
        WF1 = const.tile([97, 16], DT, tag="WF1")
        nc.sync.dma_start(WF1[:], wf1.ap())
        WF2 = const.tile([16, 1], DT, tag="WF2")
        nc.sync.dma_start(WF2[:], wf2.ap())
        BF2 = const.tile([1, 1], F32, tag="BF2")
        nc.sync.dma_start(BF2[:], bf2.ap())

        S = const.tile([97, BL], DT, tag="S")
        C = const.tile([HS, BL], CDT, tag="C")
        nc.vector.memset(S[:], 0.0)
        nc.vector.memset(S[96:97, :], 1.0)
        nc.vector.memset(C[:], 0.0)

        # Per-gate PSUM tiles (per-bank dependency tracking, so each
        # activation op starts as soon as its own gate's matmuls finish):
        # Pf [96,512] (f), Pio [96,1024] (i|o), Pg [96,512] (g).
        # Layer-1 (cols 0:64 of each gate's weight block) and layer-2
        # (cols 64:96) are fused into ONE M=96 matmul per gate — they
        # share the rhs S[0:97].  The x-projection is a K=4 matmul per
        # gate (M=96, layer-2 columns zero) emitted one step AHEAD
        # (start=True), so it fills the TensorE pipe during the previous
        # step's ACT/DVE chain; the recurrent matmul accumulates on top.
        #
        # Software-pipelined over layers: at iteration k the layer-1
        # partition computes LSTM-1 step k while the layer-2 partition
        # computes LSTM-2 step k-1 (both read h1_{k-1} from S).
        # Iteration 0 produces garbage layer-2 state (cleared after);
        # iteration n_steps produces garbage layer-1 state (the head
        # weights are zero over the h1 rows).
        GSEL = {"i": 0, "f": 1, "g": 2, "o": 3}

        def alloc_P():
            Pf = psum.tile([HS, BL], F32, tag="Pf")
            Pio = psum.tile([HS, 2 * BL], F32, tag="Pio")
            Pg = psum.tile([HS, BL], F32, tag="Pg")
            # (gate, dest-ap) in emission order: f, i, o, g
            return [
                ("f", Pf[:, :]),
                ("i", Pio[:, 0:BL]),
                ("o", Pio[:, BL:]),
                ("g", Pg[:, :]),
            ], Pf, Pio, Pg

        def emit_x_mms(banks, step):
            XTT = xpool.tile([4, BL], DT, tag="xt")
            nc.sync.dma_start(XTT[:], xT.ap()[4 * step : 4 * step + 4, :])
            for gate, dest in banks:
                gsel = GSEL[gate]
                nc.tensor.matmul(
                    dest,
                    W1X[:, gsel * HS : (gsel + 1) * HS],
                    XTT[:],
                    start=True,
                    stop=False,
                )

        with tc.tile_pool(name="psum", bufs=2, space="PSUM") as psum:
            banks, Pf, Pio, Pg = alloc_P()
            emit_x_mms(banks, 0)
            for k in range(n_steps + 1):
                has_x = k < n_steps  # P already holds the x contribution
                for gate, dest in banks:
                    gsel = GSEL[gate]
                    nc.tensor.matmul(
                        dest,
                        W12[:, gsel * HS : (gsel + 1) * HS],
                        S[0:97, :],
                        start=not has_x,
                        stop=True,
                    )

                if k + 1 <= n_steps:
                    nbanks, nPf, nPio, nPg = alloc_P()
                    if k + 1 < n_steps:
                        emit_x_mms(nbanks, k + 1)

                SIGF = gates.tile([HS, BL], DT, tag="SIGF")
                SIGIO = gates.tile([HS, 2 * BL], DT, tag="SIGIO")
                G = gates.tile([HS, BL], DT, tag="G")
                nc.scalar.activation(SIGF[:], Pf[:, :], AF.Sigmoid)
                nc.scalar.activation(SIGIO[:], Pio[:, :], AF.Sigmoid)
                nc.scalar.activation(G[:], Pg[:, :], AF.Tanh)

                U = gates.tile([HS, BL], DT, tag="U")
                V = gates.tile([HS, BL], CDT, tag="V")
                nc.vector.tensor_mul(V[:], SIGF[:], C[:])               # f*c
                nc.vector.tensor_mul(U[:], SIGIO[:, 0:BL], G[:])        # i*g
                nc.vector.tensor_add(C[:], U[:], V[:])                  # c'
                TC = gates.tile([HS, BL], DT, tag="TC")
                nc.scalar.activation(TC[:], C[:], AF.Tanh)
                nc.vector.tensor_mul(S[0:HS, :], SIGIO[:, BL:], TC[:])  # h
                if k == 0:
                    # wipe the garbage layer-2 state from the pipeline warmup
                    nc.vector.memset(S[H1:HS, :], 0.0)
                    nc.vector.memset(C[H1:HS, :], 0.0)
                if k + 1 <= n_steps:
                    banks, Pf, Pio, Pg = nbanks, nPf, nPio, nPg

        # MLP head on h2 at the last timestep (rows 64:96 of S).
        with tc.tile_pool(name="psum_head", bufs=1, space="PSUM") as psh:
            PF = psh.tile([16, BL], F32, tag="PF")
            nc.tensor.matmul(PF[:], WF1[:, :], S[0:97, :], start=True, stop=True)
            Z = gates.tile([16, BL], DT, tag="Z")
            nc.scalar.activation(Z[:], PF[:], AF.Relu)
            PO = psh.tile([1, BL], F32, tag="PO")
            nc.tensor.matmul(PO[:], WF2[:, :], Z[:], start=True, stop=True)
            Y = gates.tile([1, BL], F32, tag="Y")
            nc.scalar.activation(Y[:], PO[:], AF.Identity, bias=BF2[:, 0:1])
            nc.sync.dma_start(out.ap(), Y[:])

    nc.compile()
    return nc


def _pack_weights(inputs, np_dt):
    w_ih1, w_hh1 = inputs["w_ih1"], inputs["w_hh1"]
    w_ih2, w_hh2 = inputs["w_ih2"], inputs["w_hh2"]
    b1 = (inputs["b_ih1"] + inputs["b_hh1"]).astype(np.float32)
    b2 = (inputs["b_ih2"] + inputs["b_hh2"]).astype(np.float32)
    # Layer-1 gate weights as [97, 256]: rows = [w_hh1^T(64); zeros(32);
    # bias1(1)] matching rhs S[0:97] = [h1; h2(ignored); ones].
    z32 = np.zeros((4 * H1, 32), np.float32)
    w1t = np.concatenate([w_hh1, z32, b1[:, None]], axis=1).T
    # Layer-2 gate weights as [97, 128]: rows = [w_ih2^T(64); w_hh2^T(32);
    # bias2(1)].
    w2t = np.concatenate([w_ih2, w_hh2, b2[:, None]], axis=1).T
    # Fused per-gate blocks [97, 96]: layer-1 output units in cols 0:64,
    # layer-2 in cols 64:96 (one M=96 matmul per gate).
    w12t = np.concatenate(
        [np.concatenate([w1t[:, g * H1 : (g + 1) * H1],
                         w2t[:, g * H2 : (g + 1) * H2]], axis=1)
         for g in range(4)], axis=1)
    # Input projection [4, 384]: per gate [w_ih1^T (64) | zeros (32)].
    zx = np.zeros((4, H2), np.float32)
    w1x = np.concatenate(
        [np.concatenate([w_ih1.T[:, g * H1 : (g + 1) * H1], zx], axis=1)
         for g in range(4)], axis=1)
    return {
        "w12t": np.ascontiguousarray(w12t).astype(np_dt),
        "w1x": np.ascontiguousarray(w1x).astype(np_dt),
        "wf1": np.ascontiguousarray(np.concatenate(
            [np.zeros((64, 16), np.float32), inputs["w_fc1"].T,
             inputs["b_fc1"][None, :]], axis=0)).astype(np_dt),
        "wf2": np.ascontiguousarray(inputs["w_fc2"].T).astype(np_dt),
        "bf2": np.ascontiguousarray(inputs["b_fc2"][:, None]).astype(np.float32),
    }


_built = {}


def _get_nc(n_steps):
    if n_steps not in _built:
        _built[n_steps] = _build(n_steps)
    return _built[n_steps]


def _run(inputs, n_steps=T, **run_kwargs):
    np_dt = mybir.dt.np(DT)
    x = np.asarray(inputs["x"], np.float32)
    nb = x.shape[0]
    ncores = NCORES
    bl = nb // ncores
    assert bl == BL and x.shape[1] >= n_steps
    shared = _pack_weights({k: np.asarray(v, np.float32) for k, v in inputs.items()
                            if k != "x"} | {}, np_dt)
    in_maps = []
    for c in range(ncores):
        xs = x[c * bl : (c + 1) * bl, :n_steps, :]  # [BL, T, 4]
        xT = np.ascontiguousarray(xs.transpose(1, 2, 0).reshape(n_steps * 4, bl))
        in_maps.append(dict(shared, xT=xT.astype(np_dt)))
    nc = _get_nc(n_steps)
    res = bass_utils.run_bass_kernel_spmd(
        nc, in_maps, core_ids=list(range(ncores)), **run_kwargs
    )
    y = np.concatenate(
        [np.asarray(r["out"], np.float32).reshape(bl, 1) for r in res.results], axis=0
    )
    return y, res


def kernel(**inputs) -> np.ndarray:
    y, _ = _run(inputs)
    return y


# revision 17
# speedup vs baseline: 5.9937x; 1.1489x over previous
"""Trainium2 Bass kernel for a 2-layer LSTM (64, 32) + MLP head.

Model (PyTorch semantics, eval mode):
    h1 = LSTM(4 -> 64)(x)            x: [B=4096, T=512, 4]
    h2 = LSTM(64 -> 32)(h1)
    y  = (relu(h2[:, -1] @ w_fc1.T + b_fc1)) @ w_fc2.T + b_fc2   # [B, 1]

Sharding: data-parallel over batch across 8 NeuronCores (512 rows each),
weights replicated. Inside each core the state is kept *transposed*
([units, batch]) so the per-timestep recurrent matmuls have batch on the
moving free dimension (N=512) and the gate nonlinearities run as a few
wide ops on full 96-partition stacks (layer-1 and layer-2 gates stacked).

State tile S [97, 512]: rows 0:64 = h1^T, rows 64:96 = h2^T, row 96 =
ones (bias row).  Both layers' recurrent matmuls use rhs S[0:97] (base
partition 0 — the PE moving operand must start at 0 to span >32
partitions); layer-1's weight rows over the h2 region are zeros, and
layer-2's over nothing (it genuinely uses h1+h2).  Biases ride the
ones-row through the matmul (incl. the fc1 bias in the head).
The input projection is a separate K=4 matmul per gate accumulating into
the same PSUM bank; x_t arrives per step by DMA into a small [4, 512]
rotating tile (x is recurrence-independent, so these prefetch ahead).

PSUM gate tile P [96, 2048] (4 banks): free slices i@0, f@512, o@1024,
g@1536; partitions 0:64 = layer-1 gate, 64:96 = layer-2 gate.  Sigmoid
is then ONE activation op over [96, 1536] (i,f,o) and tanh one op over
[96, 512] (g); the cell/hidden updates are [96, 512] vector ops.
"""

import numpy as np
from contextlib import ExitStack

import concourse.bass as bass
import concourse.tile as tile
from concourse import bacc, mybir
from concourse import bass_utils

AF = mybir.ActivationFunctionType

B, T, D_IN, H1, H2 = 4096, 512, 4, 64, 32
NCORES = 8
BL = B // NCORES  # 512 batch rows per core

F32 = mybir.dt.float32
# Compute dtypes (flip for perf/accuracy trades):
DT = mybir.dt.bfloat16  # weights / state / gate-activation dtype
CDT = mybir.dt.bfloat16  # cell-state dtype

HS = H1 + H2  # 96: stacked (layer1, layer2) partition extent


def _build(n_steps: int = T):
    """Build the SPMD single-core Bass program (same NEFF on all 8 cores)."""
    nc = bacc.Bacc("TRN2", target_bir_lowering=False, debug=False)

    xT = nc.dram_tensor("xT", [n_steps * 4, BL], DT, kind="ExternalInput")
    w12t = nc.dram_tensor("w12t", [97, 4 * HS], DT, kind="ExternalInput")
    w1x = nc.dram_tensor("w1x", [4, 4 * HS], DT, kind="ExternalInput")
    wf1 = nc.dram_tensor("wf1", [97, 16], DT, kind="ExternalInput")
    wf2 = nc.dram_tensor("wf2", [16, 1], DT, kind="ExternalInput")
    bf2 = nc.dram_tensor("bf2", [1, 1], F32, kind="ExternalInput")
    out = nc.dram_tensor("out", [1, BL], F32, kind="ExternalOutput")

    with tile.TileContext(nc) as tc, ExitStack() as ctx:
        const = ctx.enter_context(tc.tile_pool(name="const", bufs=1))
        xpool = ctx.enter_context(tc.tile_pool(name="xp", bufs=8))
        gates = ctx.enter_context(tc.tile_pool(name="gates", bufs=3))

        W12 = const.tile([97, 4 * HS], DT, tag="W12")
        nc.sync.dma_start(W12[:], w12t.ap())
        W1X = const.tile([4, 4 * HS], DT, tag="W1X")
        nc.sync.dma_start(W1X[:], w1x.ap())
        WF1 = const.tile([97, 16], DT, tag="WF1")
        nc.sync.dma_start(WF1[:], wf1.ap())
        WF2 = const.tile([16, 1], DT, tag="WF2")
        nc.sync.dma_start(WF2[:], wf2.ap())
        BF2 = const.tile([1, 1], F32, tag="BF2")
        nc.sync.dma_start(BF2[:], bf2.ap())

        S = const.tile([97, BL], DT, tag="S")
        C = const.tile([HS, BL], CDT, tag="C")
        nc.vector.memset(S[:], 0.0)
        nc.vector.memset(S[96:97, :], 1.0)
        nc.vector.memset(C[:], 0.0)

        # Per-gate PSUM tiles (per-bank dependency tracking, so each
        # activation op starts as soon as its own gate's matmuls finish):
        # Pf [96,512] (f), Pio [96,1024] (i|o), Pg [96,512] (g).
        # Layer-1 (cols 0:64 of each gate's weight block) and layer-2
        # (cols 64:96) are fused into ONE M=96 matmul per gate — they
        # share the rhs S[0:97].  The x-projection is a K=4 matmul per
        # gate (M=96, layer-2 columns zero) emitted one step AHEAD
        # (start=True), so it fills the TensorE pipe during the previous
        # step's ACT/DVE chain; the recurrent matmul accumulates on top.
        #
        # Software-pipelined over layers: at iteration k the layer-1
        # partition computes LSTM-1 step k while the layer-2 partition
        # computes LSTM-2 step k-1 (both read h1_{k-1} from S).
        # Iteration 0 produces garbage layer-2 state (cleared after);
        # iteration n_steps produces garbage layer-1 state (the head
        # weights are zero over the h1 rows).
        GSEL = {"i": 0, "f": 1, "g": 2, "o": 3}

        def alloc_P():
            Pf = psum.tile([HS, BL], F32, tag="Pf")
            Pi = psum.tile([HS, BL], F32, tag="Pi")
            Pg = psum.tile([HS, BL], F32, tag="Pg")
            Po = psum.tile([HS, BL], F32, tag="Po")
            # (gate, dest-ap) in emission order: f, i, g, o —
            # f first (feeds f*c as early as possible), o last (only
            # needed at the very end for h = o * tanh(c)).
            return [
                ("f", Pf[:, :]),
                ("i", Pi[:, :]),
                ("g", Pg[:, :]),
                ("o", Po[:, :]),
            ], Pf, Pi, Pg, Po

        def emit_x_mms(banks, step):
            XTT = xpool.tile([4, BL], DT, tag="xt")
            nc.sync.dma_start(XTT[:], xT.ap()[4 * step : 4 * step + 4, :])
            for gate, dest in banks:
                gsel = GSEL[gate]
                nc.tensor.matmul(
                    dest,
                    W1X[:, gsel * HS : (gsel + 1) * HS],
                    XTT[:],
                    start=True,
                    stop=False,
                )

        with tc.tile_pool(name="psum", bufs=2, space="PSUM") as psum:
            banks, Pf, Pi, Pg, Po = alloc_P()
            emit_x_mms(banks, 0)
            for k in range(n_steps + 1):
                has_x = k < n_steps  # P already holds the x contribution
                for gate, dest in banks:
                    gsel = GSEL[gate]
                    nc.tensor.matmul(
                        dest,
                        W12[:, gsel * HS : (gsel + 1) * HS],
                        S[0:97, :],
                        start=not has_x,
                        stop=True,
                    )

                if k + 1 <= n_steps:
                    nbanks, nPf, nPi, nPg, nPo = alloc_P()
                    if k + 1 < n_steps:
                        emit_x_mms(nbanks, k + 1)

                SIGF = gates.tile([HS, BL], DT, tag="SIGF")
                SIGI = gates.tile([HS, BL], DT, tag="SIGI")
                G = gates.tile([HS, BL], DT, tag="G")
                SIGO = gates.tile([HS, BL], DT, tag="SIGO")
                nc.scalar.activation(SIGF[:], Pf[:, :], AF.Sigmoid)
                nc.scalar.activation(SIGI[:], Pi[:, :], AF.Sigmoid)
                nc.scalar.activation(G[:], Pg[:, :], AF.Tanh)
                nc.scalar.activation(SIGO[:], Po[:, :], AF.Sigmoid)

                U = gates.tile([HS, BL], DT, tag="U")
                V = gates.tile([HS, BL], CDT, tag="V")
                nc.vector.tensor_mul(V[:], SIGF[:], C[:])               # f*c
                nc.vector.tensor_mul(U[:], SIGI[:], G[:])               # i*g
                nc.vector.tensor_add(C[:], U[:], V[:])                  # c'
                TC = gates.tile([HS, BL], DT, tag="TC")
                nc.scalar.activation(TC[:], C[:], AF.Tanh)
                nc.vector.tensor_mul(S[0:HS, :], SIGO[:], TC[:])        # h
                if k == 0:
                    # wipe the garbage layer-2 state from the pipeline warmup
                    nc.vector.memset(S[H1:HS, :], 0.0)
                    nc.vector.memset(C[H1:HS, :], 0.0)
                if k + 1 <= n_steps:
                    banks, Pf, Pi, Pg, Po = nbanks, nPf, nPi, nPg, nPo

        # MLP head on h2 at the last timestep (rows 64:96 of S).
        with tc.tile_pool(name="psum_head", bufs=1, space="PSUM") as psh:
            PF = psh.tile([16, BL], F32, tag="PF")
            nc.tensor.matmul(PF[:], WF1[:, :], S[0:97, :], start=True, stop=True)
            Z = gates.tile([16, BL], DT, tag="Z")
            nc.scalar.activation(Z[:], PF[:], AF.Relu)
            PO = psh.tile([1, BL], F32, tag="PO")
            nc.tensor.matmul(PO[:], WF2[:, :], Z[:], start=True, stop=True)
            Y = gates.tile([1, BL], F32, tag="Y")
            nc.scalar.activation(Y[:], PO[:], AF.Identity, bias=BF2[:, 0:1])
            nc.sync.dma_start(out.ap(), Y[:])

    nc.compile()
    return nc


def _pack_weights(inputs, np_dt):
    w_ih1, w_hh1 = inputs["w_ih1"], inputs["w_hh1"]
    w_ih2, w_hh2 = inputs["w_ih2"], inputs["w_hh2"]
    b1 = (inputs["b_ih1"] + inputs["b_hh1"]).astype(np.float32)
    b2 = (inputs["b_ih2"] + inputs["b_hh2"]).astype(np.float32)
    # Layer-1 gate weights as [97, 256]: rows = [w_hh1^T(64); zeros(32);
    # bias1(1)] matching rhs S[0:97] = [h1; h2(ignored); ones].
    z32 = np.zeros((4 * H1, 32), np.float32)
    w1t = np.concatenate([w_hh1, z32, b1[:, None]], axis=1).T
    # Layer-2 gate weights as [97, 128]: rows = [w_ih2^T(64); w_hh2^T(32);
    # bias2(1)].
    w2t = np.concatenate([w_ih2, w_hh2, b2[:, None]], axis=1).T
    # Fused per-gate blocks [97, 96]: layer-1 output units in cols 0:64,
    # layer-2 in cols 64:96 (one M=96 matmul per gate).
    w12t = np.concatenate(
        [np.concatenate([w1t[:, g * H1 : (g + 1) * H1],
                         w2t[:, g * H2 : (g + 1) * H2]], axis=1)
         for g in range(4)], axis=1)
    # Input projection [4, 384]: per gate [w_ih1^T (64) | zeros (32)].
    zx = np.zeros((4, H2), np.float32)
    w1x = np.concatenate(
        [np.concatenate([w_ih1.T[:, g * H1 : (g + 1) * H1], zx], axis=1)
         for g in range(4)], axis=1)
    return {
        "w12t": np.ascontiguousarray(w12t).astype(np_dt),
        "w1x": np.ascontiguousarray(w1x).astype(np_dt),
        "wf1": np.ascontiguousarray(np.concatenate(
            [np.zeros((64, 16), np.float32), inputs["w_fc1"].T,
             inputs["b_fc1"][None, :]], axis=0)).astype(np_dt),
        "wf2": np.ascontiguousarray(inputs["w_fc2"].T).astype(np_dt),
        "bf2": np.ascontiguousarray(inputs["b_fc2"][:, None]).astype(np.float32),
    }


_built = {}


def _get_nc(n_steps):
    if n_steps not in _built:
        _built[n_steps] = _build(n_steps)
    return _built[n_steps]


def _run(inputs, n_steps=T, **run_kwargs):
    np_dt = mybir.dt.np(DT)
    x = np.asarray(inputs["x"], np.float32)
    nb = x.shape[0]
    ncores = NCORES
    bl = nb // ncores
    assert bl == BL and x.shape[1] >= n_steps
    shared = _pack_weights({k: np.asarray(v, np.float32) for k, v in inputs.items()
                            if k != "x"} | {}, np_dt)
    in_maps = []
    for c in range(ncores):
        xs = x[c * bl : (c + 1) * bl, :n_steps, :]  # [BL, T, 4]
        xT = np.ascontiguousarray(xs.transpose(1, 2, 0).reshape(n_steps * 4, bl))
        in_maps.append(dict(shared, xT=xT.astype(np_dt)))
    nc = _get_nc(n_steps)
    res = bass_utils.run_bass_kernel_spmd(
        nc, in_maps, core_ids=list(range(ncores)), **run_kwargs
    )
    y = np.concatenate(
        [np.asarray(r["out"], np.float32).reshape(bl, 1) for r in res.results], axis=0
    )
    return y, res


def kernel(**inputs) -> np.ndarray:
    y, _ = _run(inputs)
    return y
